# revision 3
# baseline (speedup 1.0000x reference)
"""Trainium2 Bass kernel for nn_Diffuser (sparse_attention) — v4.

Algebra (no softmax): mean_r y_rh = q_h G_h (q_h^T v_h) with
G_h = s^2/R * sum_r k_rh^T k_rh, k = ref @ Wk.  The O(N^3) chain is gone.

v4 over v3:
 - All inputs/outputs host-swizzled to partition-major [128, X] layouts:
   every DMA is one maximal-run descriptor per partition.
 - Attention emission reordered: all qg/w matmuls first (drains chase),
   then all z matmuls — removes the per-pair PE drain stall.
 - z matmuls emitted twice (both partition halves) so the scramble's
   duplication DMAs disappear entirely.
 - gamma/beta/bproj packed into one [128, 18] input.
"""

import numpy as np
from contextlib import ExitStack

import concourse.bass as bass
import concourse.tile as tile
from concourse import bacc, mybir
from concourse.bass_utils import run_bass_kernel_spmd
from concourse.masks import make_identity

F32 = mybir.dt.float32
F32R = mybir.dt.float32r
BF16 = mybir.dt.bfloat16
AF = mybir.ActivationFunctionType
ALU = mybir.AluOpType

D = 768
H = 12
HD = 64
R = 10
N = 256
STEPS = 3
NB = 8
CC = D // 128
PAIRS = H // 2
SCALE = HD ** -0.5
EPS = 1e-5
GS = SCALE * SCALE / R
SQRT_GS = float(np.sqrt(GS))

RCH = 3  # ref chunks per core (20 real + 4 zero-pad = 24)


def _emit(nc, tc, ctx, t_xT, t_refT, t_Wqv, t_Wk, t_Wproj, t_gbb, t_out,
          g_in, g_out, iters=1):
    const = ctx.enter_context(tc.tile_pool(name="const", bufs=1))
    persist = ctx.enter_context(tc.tile_pool(name="persist", bufs=1))

    ident_bf = const.tile([128, 128], BF16)
    make_identity(nc, ident_bf)
    ones_f32 = const.tile([128, 128], F32)
    nc.gpsimd.memset(ones_f32, 1.0)
    ones_f = const.tile([128, 128], F32R)
    nc.scalar.copy(ones_f[:], ones_f32[:])
    eps_sb = const.tile([128, 1], F32)
    nc.gpsimd.memset(eps_sb, EPS)

    gbb_sb = const.tile([128, 3 * CC], F32)

    Wk_sb = persist.tile([128, CC, D], BF16, name="Wk_sb")
    refT_sb = persist.tile([128, CC, RCH * 128], BF16, name="refT_sb")
    xT = persist.tile([128, CC, N], F32R, name="xT")
    out_f32 = persist.tile([128, CC, N], F32, name="out_f32")
    Wqv_sb = persist.tile([128, CC, 2 * D], F32R, name="Wqv_sb")
    Wproj_sb = persist.tile([128, CC, D], F32R, name="Wproj_sb")

    k_bf = persist.tile([128, RCH, D], BF16, name="k_bf")
    gpart = persist.tile([128, PAIRS * 128], F32, name="gpart")
    G_bf = persist.tile([128, PAIRS * 128], BF16, name="G_bf")
    gsum = persist.tile([128, PAIRS * 128], F32, name="gsum")

    qv_bf = persist.tile([128, 2, 2 * D], BF16, name="qv_bf")
    qT_bf = persist.tile([128, CC, N], BF16, name="qT_bf")
    w_bf = persist.tile([128, PAIRS, 128], BF16, name="w_bf")
    qg_bf = persist.tile([128, PAIRS, N], BF16, name="qg_bf")
    m_sb = persist.tile([128, H * N], F32, name="m_sb")
    zT_sb = persist.tile([128, CC, N], F32R, name="zT_sb")
    xp_sb = persist.tile([128, CC, N], F32R, name="xp_sb")
    sq_sb = persist.tile([128, CC, N], F32R, name="sq_sb")

    mean_b = persist.tile([128, N], F32, name="mean_b")
    mean2_b = persist.tile([128, N], F32, name="mean2_b")
    var_b = persist.tile([128, N], F32, name="var_b")
    rsig_b = persist.tile([128, N], F32, name="rsig_b")
    tmp_b = persist.tile([128, CC, N], F32, name="tmp_b")
    tmp2_b = persist.tile([128, CC, N], F32, name="tmp2_b")

    def gamma_ap(mc):
        return gbb_sb[:, mc:mc + 1]

    def beta_ap(mc):
        return gbb_sb[:, CC + mc:CC + mc + 1]

    def bproj_ap(mc):
        return gbb_sb[:, 2 * CC + mc:2 * CC + mc + 1]

    def one_round(it, first, parts=("dma", "gbar", "steps")):
        if "dma" in parts:
            nc.sync.dma_start(
                out=refT_sb,
                in_=t_refT.ap().rearrange("p (c m) -> p c m", c=CC))
            nc.sync.dma_start(
                out=Wk_sb, in_=t_Wk.ap().rearrange("p (c d) -> p c d", c=CC))
            nc.sync.dma_start(
                out=xT, in_=t_xT.ap().rearrange("p (c n) -> p c n", c=CC))
            nc.sync.dma_start(
                out=Wqv_sb, in_=t_Wqv.ap().rearrange("p (c d) -> p c d", c=CC))
            nc.sync.dma_start(
                out=Wproj_sb,
                in_=t_Wproj.ap().rearrange("p (c d) -> p c d", c=CC))
            nc.sync.dma_start(out=gbb_sb, in_=t_gbb.ap())

        if "gbar" in parts:
            emit_gbar(it, first)

        if "steps" in parts:
            emit_steps(it, first, parts)

    def emit_gbar(it, first):
        with tc.tile_pool(name=f"kps{it}", bufs=2, space="PSUM") as kps, \
             tc.tile_pool(name=f"gps{it}", bufs=1, space="PSUM") as gps:
            for c in range(RCH):
                for si, (o, wd) in enumerate(((0, 512), (512, 256))):
                    pt = kps.tile([128, wd], F32, tag=f"k{si}",
                                  name=f"kp{it}_{c}_{si}")
                    for kc in range(CC):
                        nc.tensor.matmul(
                            pt[:],
                            refT_sb[:, kc, c * 128:(c + 1) * 128],
                            Wk_sb[:, kc, o:o + wd],
                            start=(kc == 0), stop=(kc == CC - 1))
                    dst = k_bf[:, c, o:o + wd]
                    if (c + si) % 2 == 0:
                        nc.scalar.activation(dst, pt[:], AF.Copy,
                                             scale=SQRT_GS)
                    else:
                        nc.vector.tensor_scalar_mul(dst, pt[:], SQRT_GS)
            # one zero-region (2KB bank) covers 4 pair-blocks: start only on
            # the first write into each bank, stop on the last
            gp = gps.tile([128, PAIRS * 128], F32, tag="g", name=f"gp{it}")
            for p in range(PAIRS):
                first_in_bank = p % 4 == 0
                last_in_bank = (p % 4 == 3) or (p == PAIRS - 1)
                for c in range(RCH):
                    nc.tensor.matmul(
                        gp[:, p * 128:(p + 1) * 128],
                        k_bf[:, c, p * 128:(p + 1) * 128],
                        k_bf[:, c, p * 128:(p + 1) * 128],
                        start=(first_in_bank and c == 0),
                        stop=(last_in_bank and c == RCH - 1),
                        skip_group_check=True)
            for half in range(2):
                dst = gpart[:, half * 384:(half + 1) * 384]
                src = gp[:, half * 384:(half + 1) * 384]
                if half == 0:
                    nc.vector.tensor_copy(dst, src)
                else:
                    nc.scalar.copy(dst, src)
        nc.sync.dma_start(out=g_in.ap(), in_=gpart[:])
        if first:
            # NRT requires collectives in straight-line order: emitted once,
            # outside any hardware loop
            nc.gpsimd.collective_compute(
                "AllReduce", ALU.add,
                replica_groups=[list(range(NB))],
                ins=[g_in[:]], outs=[g_out[:]])

    def emit_steps(it, first, parts):
        if not any(p in parts for p in ("qv", "qt", "attn", "scram", "proj", "ln", "none")):
            parts = parts + ("qv", "qt", "attn", "scram", "proj", "ln")
        for step in range(STEPS):
            # ---- qv natural: lhsT (xT block) stationary across supertiles ----
            with tc.tile_pool(name=f"qvp{it}_{step}", bufs=2, space="PSUM") as qvp:
              if "qv" not in parts:
                nc.scalar.copy(qv_bf[:, 0, 0:1], xT[:, 0, 0:1].bitcast(F32))
              else:
                for nch in range(2):
                    for sup in range(3):
                        pt = qvp.tile([128, 512], F32, tag=f"qv{sup}",
                                      name=f"qvp{it}_{step}_{nch}_{sup}")
                        for kc in range(CC):
                            nc.tensor.matmul(
                                pt[:],
                                xT[:, kc, nch * 128:(nch + 1) * 128],
                                Wqv_sb[:, kc, sup * 512:(sup + 1) * 512],
                                start=(kc == 0), stop=(kc == CC - 1))
                        dst = qv_bf[:, nch, sup * 512:(sup + 1) * 512]
                        if (nch + sup) % 2 == 0:
                            nc.scalar.copy(dst, pt[:])
                        else:
                            nc.vector.tensor_copy(dst, pt[:])

            # ---- qT: transpose q (bf16) back to c-major ----
            with tc.tile_pool(name=f"qtp{it}_{step}", bufs=2, space="PSUM") as qtp:
              if "qt" not in parts:
                nc.scalar.copy(qT_bf[:, 0, 0:1], qv_bf[:, 0, 0:1])
              else:
                for cc in range(CC):
                    pt = qtp.tile([128, N], BF16, tag="qt",
                                  name=f"qt{it}_{step}_{cc}")
                    for nch in range(2):
                        nc.tensor.matmul(
                            pt[:, nch * 128:(nch + 1) * 128],
                            qv_bf[:, nch, cc * 128:(cc + 1) * 128],
                            ident_bf[:],
                            is_transpose=True, skip_group_check=True,
                            start=(nch == 0), stop=(nch == 1))
                    if cc % 2 == 0:
                        nc.vector.tensor_copy(qT_bf[:, cc, :], pt[:])
                    else:
                        nc.scalar.copy(qT_bf[:, cc, :], pt[:])

                if step == 0 and first:
                    # fetch the collective result here: after step-0 qv/qT so
                    # the wait doesn't head-of-line-block the engine queues
                    nc.sync.dma_start(out=gsum[:], in_=g_out.ap())
                    nc.scalar.copy(G_bf[:], gsum[:])

            # ---- attention: qg/w batch first, z batch second ----
            with tc.tile_pool(name=f"att{it}_{step}", bufs=2, space="PSUM") as att:
              if "attn" not in parts:
                nc.scalar.copy(m_sb[:, 0:1], qT_bf[:, 0, 0:1])
              else:
                for p in range(PAIRS):
                    qgp = att.tile([128, N], F32, tag="qg",
                                   name=f"qg{it}_{step}_{p}")
                    for par in range(2):
                        nc.tensor.matmul(
                            qgp[par * 64:(par + 1) * 64, :],
                            G_bf[par * 64:(par + 1) * 64,
                                 p * 128 + par * 64:p * 128 + (par + 1) * 64],
                            qT_bf[par * 64:(par + 1) * 64, p, :],
                            skip_group_check=True)
                    wp = att.tile([128, 128], F32, tag="w",
                                  name=f"w{it}_{step}_{p}")
                    for nch in range(2):
                        nc.tensor.matmul(
                            wp[:],
                            qv_bf[:, nch, p * 128:(p + 1) * 128],
                            qv_bf[:, nch, D + p * 128:D + (p + 1) * 128],
                            start=(nch == 0), stop=(nch == 1))
                    if p % 2 == 0:
                        nc.vector.tensor_copy(qg_bf[:, p, :], qgp[:])
                        nc.scalar.copy(w_bf[:, p, :], wp[:])
                    else:
                        nc.scalar.copy(qg_bf[:, p, :], qgp[:])
                        nc.vector.tensor_copy(w_bf[:, p, :], wp[:])
                for p in range(PAIRS):
                    # z written to BOTH partition halves (two matmuls per
                    # head) so no duplication DMAs are needed for the
                    # scramble; m_sb col = 256*h + n
                    zpA = att.tile([128, N], F32, tag="zA",
                                   name=f"zA{it}_{step}_{p}")
                    zpB = att.tile([128, N], F32, tag="zB",
                                   name=f"zB{it}_{step}_{p}")
                    for half in range(2):
                        nc.tensor.matmul(
                            zpA[half * 64:(half + 1) * 64, :],
                            w_bf[0:64, p, 0:64],
                            qg_bf[0:64, p, :],
                            skip_group_check=True,
                            tile_position=(0, half * 64))
                        nc.tensor.matmul(
                            zpB[half * 64:(half + 1) * 64, :],
                            w_bf[64:128, p, 64:128],
                            qg_bf[64:128, p, :],
                            skip_group_check=True,
                            tile_position=(64, half * 64))
                    if p % 2 == 0:
                        nc.scalar.copy(
                            m_sb[:, (2 * p) * N:(2 * p + 1) * N], zpA[:])
                        nc.vector.tensor_copy(
                            m_sb[:, (2 * p + 1) * N:(2 * p + 2) * N], zpB[:])
                    else:
                        nc.vector.tensor_copy(
                            m_sb[:, (2 * p) * N:(2 * p + 1) * N], zpA[:])
                        nc.scalar.copy(
                            m_sb[:, (2 * p + 1) * N:(2 * p + 2) * N], zpB[:])

            # ---- scramble: stride-12 regather ----
            if "scram" not in parts:
                nc.scalar.copy(zT_sb[:, 0, 0:1], m_sb[:, 0:1])
            if "scram" in parts:
                for cc in range(CC):
                    if cc % 3 == 2:
                        nc.gpsimd.tensor_copy(zT_sb[0:64, cc, :],
                                              m_sb[0:64, 2 * cc::12])
                    else:
                        nc.vector.tensor_copy(zT_sb[0:64, cc, :],
                                              m_sb[0:64, 2 * cc::12])
                    if cc % 3 == 1:
                        nc.gpsimd.tensor_copy(zT_sb[64:128, cc, :],
                                              m_sb[64:128, 2 * cc + 1::12])
                    else:
                        nc.scalar.copy(zT_sb[64:128, cc, :],
                                       m_sb[64:128, 2 * cc + 1::12])

            # ---- proj (kc-outer: pipelines with the gathers) + LN ----
            with tc.tile_pool(name=f"prj{it}_{step}", bufs=1, space="PSUM") as prj, \
                 tc.tile_pool(name=f"lns{it}_{step}", bufs=1, space="PSUM") as lns:
                pps = [prj.tile([128, N], F32, tag=f"xp{mc}",
                                name=f"xp{it}_{step}_{mc}") for mc in range(CC)]
                if "proj" not in parts:
                    nc.scalar.copy(xp_sb[:, 0, 0:1], zT_sb[:, 0, 0:1])
                if "proj" in parts:
                  for mc in range(CC):
                    for kc in range(CC):
                        nc.tensor.matmul(
                            pps[mc][:],
                            Wproj_sb[:, kc, mc * 128:(mc + 1) * 128],
                            zT_sb[:, kc, :],
                            start=(kc == 0), stop=(kc == CC - 1))
                  for mc in range(CC):
                    nc.scalar.activation(xp_sb[:, mc, :], pps[mc][:], AF.Identity,
                                         bias=bproj_ap(mc))
                    nc.vector.tensor_mul(sq_sb[:, mc, :],
                                         xp_sb[:, mc, :].bitcast(F32),
                                         xp_sb[:, mc, :].bitcast(F32))
                if "ln" not in parts:
                    for mc in range(CC):
                        nc.scalar.activation(xT[:, mc, 0:1], xp_sb[:, mc, 0:1],
                                             AF.Identity)
                    continue
                psum_s = lns.tile([128, N], F32, tag="s", name=f"ps{it}_{step}")
                psum_q = lns.tile([128, N], F32, tag="q", name=f"pq{it}_{step}")
                for mc in range(CC):
                    nc.tensor.matmul(psum_s[:], ones_f[:], xp_sb[:, mc, :],
                                     start=(mc == 0), stop=(mc == CC - 1))
                for mc in range(CC):
                    nc.tensor.matmul(psum_q[:], ones_f[:], sq_sb[:, mc, :],
                                     start=(mc == 0), stop=(mc == CC - 1))
                nc.scalar.activation(mean_b[:], psum_s[:], AF.Copy, scale=1.0 / D)
                nc.vector.tensor_mul(mean2_b[:], mean_b[:], mean_b[:])
                nc.vector.scalar_tensor_tensor(
                    out=var_b[:], in0=psum_q[:], scalar=1.0 / D, in1=mean2_b[:],
                    op0=ALU.mult, op1=ALU.subtract)
                nc.scalar.activation(var_b[:], var_b[:], AF.Sqrt, bias=eps_sb[:])
                nc.vector.reciprocal(rsig_b[:], var_b[:])
                last = step == STEPS - 1
                for mc in range(CC):
                    nc.vector.tensor_sub(tmp_b[:, mc, :],
                                         xp_sb[:, mc, :].bitcast(F32), mean_b[:])
                    nc.gpsimd.tensor_mul(tmp2_b[:, mc, :], tmp_b[:, mc, :],
                                         rsig_b[:])
                    dst = out_f32[:, mc, :] if last else xT[:, mc, :]
                    nc.scalar.activation(dst, tmp2_b[:, mc, :], AF.Identity,
                                         scale=gamma_ap(mc), bias=beta_ap(mc))

            if "bar" in parts:
                nc.all_engine_barrier()

        # ---- epilogue: store c-major swizzled; host untangles ----
        nc.sync.dma_start(out=t_out.ap(),
                          in_=out_f32[:].rearrange("p c n -> p (c n)"))

    import os
    loop_parts = tuple(os.environ.get("LOOP_PARTS", "dma,gbar,steps").split(","))
    if iters == 1:
        one_round(0, True)
    else:
        one_round(0, True)
        with tc.For_i(1, iters, 1):
            one_round(1, False, parts=loop_parts)


def build(iters=1):
    nc = bacc.Bacc("TRN2", target_bir_lowering=False, debug=False, num_devices=NB)
    t_xT = nc.declare_dram_parameter("xT", [128, CC * N], F32R, isOutput=False)
    t_refT = nc.declare_dram_parameter("refT", [128, CC * RCH * 128], BF16,
                                       isOutput=False)
    t_Wqv = nc.declare_dram_parameter("Wqv", [128, CC * 2 * D], F32R,
                                      isOutput=False)
    t_Wk = nc.declare_dram_parameter("Wk", [128, CC * D], BF16, isOutput=False)
    t_Wproj = nc.declare_dram_parameter("Wproj", [128, CC * D], F32R,
                                        isOutput=False)
    t_gbb = nc.declare_dram_parameter("gbb", [128, 3 * CC], F32, isOutput=False)
    t_out = nc.declare_dram_parameter("out", [128, CC * N], F32, isOutput=True)
    g_in = nc.dram_tensor("g_in", [128, PAIRS * 128], F32)
    g_out = nc.dram_tensor("g_out", [128, PAIRS * 128], F32, addr_space="Shared")
    with tile.TileContext(nc) as tc:
        with ExitStack() as ctx:
            _emit(nc, tc, ctx, t_xT, t_refT, t_Wqv, t_Wk, t_Wproj, t_gbb,
                  t_out, g_in, g_out, iters=iters)
    nc.compile()
    return nc


_CACHE = {}
last_results = None


def _swz(a, cc=CC):
    """[768, X] -> partition-major [128, cc*X]."""
    x = np.ascontiguousarray(a)
    return np.ascontiguousarray(
        x.reshape(cc, 128, -1).transpose(1, 0, 2).reshape(128, -1))


def make_in_maps(x, ref, Wqv, Wk, Wproj, bproj, gamma, beta):
    import ml_dtypes

    def f(a):
        return np.asarray(a, dtype=np.float32)

    def b16(a):
        return a.astype(ml_dtypes.bfloat16)

    x = f(x)
    refT_full = f(ref).reshape(R * N, D).T  # [768, 2560]
    gbb = np.concatenate([
        f(gamma).reshape(CC, 128).T, f(beta).reshape(CC, 128).T,
        f(bproj).reshape(CC, 128).T], axis=1)  # [128, 18]
    common = dict(Wqv=_swz(f(Wqv)), Wk=b16(_swz(f(Wk))),
                  Wproj=_swz(f(Wproj)), gbb=np.ascontiguousarray(gbb))
    in_maps = []
    for b in range(NB):
        refT = np.zeros((D, RCH * 128), np.float32)
        lo = b * RCH * 128
        hi = min(R * N, lo + RCH * 128)
        if hi > lo:
            refT[:, :hi - lo] = refT_full[:, lo:hi]
        in_maps.append(dict(xT=_swz(x[b].T), refT=b16(_swz(refT)), **common))
    return in_maps


def kernel(x, ref, Wqv, Wk, Wproj, bproj, gamma, beta):
    global last_results
    if "nc" not in _CACHE:
        _CACHE["nc"] = build()
    nc = _CACHE["nc"]
    in_maps = make_in_maps(x, ref, Wqv, Wk, Wproj, bproj, gamma, beta)
    res = run_bass_kernel_spmd(nc, in_maps, list(range(NB)))
    last_results = res
    out = []
    for b in range(NB):
        o = res.results[b]["out"]  # [128, CC*N]
        out.append(o.reshape(128, CC, N).transpose(1, 0, 2).reshape(D, N).T)
    return np.stack(out).astype(np.float32)
